# revision 15
# baseline (speedup 1.0000x reference)
"""Trainium2 Bass kernel for nn_AdditiveMask (GNN message passing, APPNP K=5).

8-core SPMD. Per core: dense tanh+normalize feature table (phase A), per-edge
cosine weights via chunked dma_gather (phase B), 5 APPNP propagation steps via
group-ELL + ap_gather + AllGather (phase C). Host does index routing only.
"""
import numpy as np

import concourse.bacc as bacc
import concourse.mybir as mybir
import concourse.tile as tile
from concourse import library_config
from concourse.bass_utils import run_bass_kernel_spmd

f32 = mybir.dt.float32
f16 = mybir.dt.float16
i16 = mybir.dt.int16

NC = 8
D = 128
N = 50000
E = 600000
import os
K = int(os.environ.get('KITER', '5'))
PHB = int(os.environ.get('PHB', '1'))
PHA = int(os.environ.get('PHA', '1'))
EPS = 1e-8

T8 = 784                   # renumbered targets per group
NLOC = 8 * T8              # 6272 padded local nodes per core
NP = NC * NLOC             # 50176 padded global nodes
ZR_HI = NP                 # zero row in hi half (>= 32768)
ZR_LO = 6271               # zero row in lo half (core0 pad slot, never assigned)
HN_ROWS = NP + 128
W8 = 8                     # ELL width
GCHUNK = 1024              # dma_gather idxs per instruction
BC = 32                    # phase-B cols per chunk (4096 slots)
NPASS = 8                  # phase-C passes

_prog_cache = {}


def _wrap16(vals, n):
    m = np.zeros((16, n // 16), np.int16)
    lin = np.arange(n)
    m[lin % 16, lin // 16] = vals.astype(np.int16)
    return np.tile(m, (8, 1))


def _wrapg(vals, slots_g):
    out = np.zeros((128, slots_g // 16), np.int16)
    lin = np.arange(slots_g)
    v = vals.reshape(8, slots_g)
    for g in range(8):
        out[16 * g + lin % 16, lin // 16] = v[g].astype(np.int16)
    return out


def _build_program(CG, O, tier_base, max_vr):
    SLOTS_G = CG * W8
    EGRID = 8 * SLOTS_G
    EC = EGRID // 128
    AF = mybir.ActivationFunctionType

    nc = bacc.Bacc("TRN2", target_bir_lowering=False, debug=False,
                   num_devices=NC)

    xT = nc.dram_tensor("xT", [128, NP], f32, kind="ExternalInput")
    WT = nc.dram_tensor("WT", [128, 128], f32, kind="ExternalInput")
    bL = nc.dram_tensor("bL", [128, 1], f32, kind="ExternalInput")
    coefs = nc.dram_tensor("coefs", [128, 4], f32, kind="ExternalInput")
    maskg = nc.dram_tensor("maskg", [128, T8], f32, kind="ExternalInput")
    hs_lo = nc.dram_tensor("hs_lo", [128, EGRID // 16], i16, kind="ExternalInput")
    hs_hi = nc.dram_tensor("hs_hi", [128, EGRID // 16], i16, kind="ExternalInput")
    hd_lo = nc.dram_tensor("hd_lo", [128, EGRID // 16], i16, kind="ExternalInput")
    hd_hi = nc.dram_tensor("hd_hi", [128, EGRID // 16], i16, kind="ExternalInput")
    apg_ix = nc.dram_tensor("apg_ix", [128, SLOTS_G // 16], i16, kind="ExternalInput")
    par_m = nc.dram_tensor("par_m", [128, SLOTS_G], f16, kind="ExternalInput")

    hn_dram = nc.dram_tensor("hn_dram", [HN_ROWS, 128], f16, kind="Internal")
    w16_dram = nc.dram_tensor("w16_dram", [1, EGRID], f16, kind="Internal")
    gblk = nc.dram_tensor("gblk", [1, NLOC], f16, kind="Internal")
    gath = nc.dram_tensor("gath", [NC, NLOC], f16, kind="Internal",
                          addr_space="Shared")
    w_out = nc.dram_tensor("w_out", [1, EGRID], f32, kind="ExternalOutput")
    fill_out = nc.dram_tensor("fill_out", [1, NLOC], f32, kind="ExternalOutput")

    with tile.TileContext(nc) as tc:
        with tc.tile_pool(name="cst", bufs=1) as cst, \
             tc.tile_pool(name="mid", bufs=1) as keep:
            WT_t = cst.tile([128, 128], f32, tag="WT")
            nc.sync.dma_start(out=WT_t[:], in_=WT.ap())
            bL_t = cst.tile([128, 1], f32, tag="bL")
            nc.sync.dma_start(out=bL_t[:], in_=bL.ap())
            co_t = cst.tile([128, 4], f32, tag="co")
            nc.sync.dma_start(out=co_t[:], in_=coefs.ap())
            from concourse.masks import make_identity
            ident = cst.tile([128, 128], f32, tag="ident")
            make_identity(nc, ident[:])
            ones1 = cst.tile([128, 1], f32, tag="ones1")
            nc.vector.memset(ones1[:], 1.0)

            # ---------- Phase A ----------
            with tc.tile_pool(name="pa", bufs=2) as wk, \
                 tc.tile_pool(name="pap", bufs=2, space="PSUM") as psp:
                for c in range(NP // 512 if PHA else 0):
                    xt_c = wk.tile([128, 512], f32, tag="xt")
                    nc.sync.dma_start(out=xt_c[:],
                                      in_=xT.ap()[:, 512 * c:512 * (c + 1)])
                    ps1 = psp.tile([128, 512], f32, tag="ps1")
                    nc.tensor.matmul(ps1[:], WT_t[:], xt_c[:])
                    hT = wk.tile([128, 512], f32, tag="hT")
                    nc.scalar.activation(hT[:], ps1[:], AF.Tanh, bias=bL_t[:, 0:1])
                    hT16 = wk.tile([128, 512], f16, tag="hT16")
                    nc.vector.tensor_copy(hT16[:], hT[:])
                    for q in range(4 if PHA >= 2 else 0):
                        h_sb = wk.tile([128, 128], f16, tag="h_sb")
                        nc.sync.dma_start_transpose(
                            h_sb[:], hT16[:, 128 * q:128 * (q + 1)])
                        if PHA >= 3:
                            sq = wk.tile([128, 128], f32, tag="sq")
                            nc.vector.tensor_mul(sq[:], h_sb[:], h_sb[:])
                            nsq = wk.tile([128, 1], f32, tag="nsq")
                            nc.vector.tensor_reduce(
                                out=nsq[:], in_=sq[:], axis=mybir.AxisListType.X,
                                op=mybir.AluOpType.add)
                            rinv = wk.tile([128, 1], f32, tag="rinv")
                            nc.scalar.activation(rinv[:], nsq[:],
                                                 AF.Abs_reciprocal_sqrt)
                        else:
                            rinv = wk.tile([128, 1], f32, tag="rinv")
                            nc.vector.memset(rinv[:], 1.0)
                        hno = wk.tile([128, 128], f16, tag="hno")
                        nc.scalar.activation(hno[:], h_sb[:], AF.Copy,
                                             scale=rinv[:, 0:1])
                        if PHA >= 4:
                            r0 = 512 * c + 128 * q
                            nc.sync.dma_start(out=hn_dram.ap()[r0:r0 + 128, :],
                                              in_=hno[:])
                zrow = wk.tile([1, 128], f16, tag="zrow")
                nc.vector.memset(zrow[:], 0.0)
                nc.sync.dma_start(out=hn_dram.ap()[ZR_HI:ZR_HI + 1, :], in_=zrow[:])
                nc.sync.dma_start(out=hn_dram.ap()[ZR_LO:ZR_LO + 1, :], in_=zrow[:])

            # ---------- Phase B ----------
            w16_all = keep.tile([128, EC], f16, tag="w16")
            with tc.tile_pool(name="pbi", bufs=1) as pbi, \
                 tc.tile_pool(name="pb", bufs=1) as wkb:
                ix_slo = pbi.tile([128, EGRID // 16], i16, tag="ix_slo")
                nc.sync.dma_start(out=ix_slo[:], in_=hs_lo.ap())
                ix_shi = pbi.tile([128, EGRID // 16], i16, tag="ix_shi")
                nc.sync.dma_start(out=ix_shi[:], in_=hs_hi.ap())
                ix_dlo = pbi.tile([128, EGRID // 16], i16, tag="ix_dlo")
                nc.sync.dma_start(out=ix_dlo[:], in_=hd_lo.ap())
                ix_dhi = pbi.tile([128, EGRID // 16], i16, tag="ix_dhi")
                nc.sync.dma_start(out=ix_dhi[:], in_=hd_hi.ap())
                hn_lo_ap = hn_dram.ap()[0:32768, :]
                hn_hi_ap = hn_dram.ap()[32768:HN_ROWS, :]
                for c in range(EC // BC if PHB else 0):
                    s0 = c * BC * 128
                    hs_t = wkb.tile([128, BC, 128], f16, tag="hs")
                    hs2_t = wkb.tile([128, BC, 128], f16, tag="hs2")
                    hd_t = wkb.tile([128, BC, 128], f16, tag="hd")
                    hd2_t = wkb.tile([128, BC, 128], f16, tag="hd2")
                    for u in range(BC * 128 // GCHUNK):
                        i0 = s0 + u * GCHUNK
                        oc = u * (GCHUNK // 128)
                        sl = slice(i0 // 16, (i0 + GCHUNK) // 16)
                        ocs = slice(oc, oc + GCHUNK // 128)
                        nc.gpsimd.dma_gather(
                            out_ap=hs_t[:, ocs, :], in_ap=hn_lo_ap,
                            idxs_ap=ix_slo[:, sl], num_idxs=GCHUNK,
                            num_idxs_reg=GCHUNK, elem_size=128)
                        nc.gpsimd.dma_gather(
                            out_ap=hs2_t[:, ocs, :], in_ap=hn_hi_ap,
                            idxs_ap=ix_shi[:, sl], num_idxs=GCHUNK,
                            num_idxs_reg=GCHUNK, elem_size=128)
                        nc.gpsimd.dma_gather(
                            out_ap=hd_t[:, ocs, :], in_ap=hn_lo_ap,
                            idxs_ap=ix_dlo[:, sl], num_idxs=GCHUNK,
                            num_idxs_reg=GCHUNK, elem_size=128)
                        nc.gpsimd.dma_gather(
                            out_ap=hd2_t[:, ocs, :], in_ap=hn_hi_ap,
                            idxs_ap=ix_dhi[:, sl], num_idxs=GCHUNK,
                            num_idxs_reg=GCHUNK, elem_size=128)
                    nc.vector.tensor_add(hs_t[:], hs_t[:], hs2_t[:])
                    nc.vector.tensor_add(hd_t[:], hd_t[:], hd2_t[:])
                    pr = wkb.tile([128, BC, 128], f32, tag="pr")
                    nc.vector.tensor_mul(pr[:], hs_t[:], hd_t[:])
                    cosn = wkb.tile([128, BC], f32, tag="cosn")
                    nc.vector.tensor_reduce(
                        out=cosn[:].rearrange("p (a o) -> p a o", o=1),
                        in_=pr[:], axis=mybir.AxisListType.X,
                        op=mybir.AluOpType.add)
                    w32c = wkb.tile([128, BC], f32, tag="w32c")
                    nc.scalar.activation(w32c[:], cosn[:], AF.Relu)
                    nc.vector.tensor_copy(w16_all[:, c * BC:(c + 1) * BC], w32c[:])
                    nc.sync.dma_start(
                        out=w_out.ap().rearrange("a (c p) -> (a p) c", p=128)
                        [:, c * BC:(c + 1) * BC], in_=w32c[:])
                if not PHB:
                    nc.vector.memset(w16_all[:], 0.25)
                nc.sync.dma_start(
                    out=w16_dram.ap().rearrange("a (c p) -> (a p) c", p=128),
                    in_=w16_all[:])

            # ---------- deg / dinv / norm ----------
            dinv = keep.tile([128, CG], f32, tag="dinv")
            norm_a = keep.tile([128, SLOTS_G], f16, tag="norm_a")  # lo part
            norm_b = keep.tile([128, SLOTS_G], f16, tag="norm_b")  # hi part
            with tc.tile_pool(name="dg", bufs=1) as dg:
                w_g = dg.tile([128, SLOTS_G], f16, tag="w_g")
                for g in range(8):
                    nc.sync.dma_start(
                        out=w_g[16 * g:16 * g + 1, :],
                        in_=w16_dram.ap()[:, g * SLOTS_G:(g + 1) * SLOTS_G])
                degr = dg.tile([128, CG], f32, tag="degr")
                nc.vector.tensor_reduce(
                    out=degr[:].rearrange("p (a o) -> p a o", o=1),
                    in_=w_g[:].rearrange("p (a w) -> p a w", w=W8),
                    axis=mybir.AxisListType.X, op=mybir.AluOpType.add)
                for r in range(2, max_vr + 1):
                    if O[r] == 0:
                        continue
                    tb = tier_base[r]
                    nc.vector.tensor_add(degr[:, 0:O[r]], degr[:, 0:O[r]],
                                         degr[:, tb:tb + O[r]])
                nc.vector.tensor_scalar_max(degr[:, 0:T8], degr[:, 0:T8], 1e-12)
                nc.scalar.activation(dinv[:, 0:T8], degr[:, 0:T8],
                                     AF.Abs_reciprocal_sqrt)
                nc.vector.memset(dinv[:, T8:CG], 0.0)
                for r in range(2, max_vr + 1):
                    if O[r] == 0:
                        continue
                    tb = tier_base[r]
                    nc.vector.tensor_copy(dinv[:, tb:tb + O[r]], dinv[:, 0:O[r]])
                parm_t = dg.tile([128, SLOTS_G], f16, tag="parm")
                nc.sync.dma_start(out=parm_t[:], in_=par_m.ap())
                nc.vector.tensor_mul(
                    norm_b[:].rearrange("p (a w) -> p a w", w=W8),
                    w_g[:].rearrange("p (a w) -> p a w", w=W8),
                    dinv[:].rearrange("p (a o) -> p a o", o=1)
                    .to_broadcast([128, CG, W8]))
                nc.vector.tensor_mul(norm_a[:], norm_b[:], parm_t[:])
                nc.vector.tensor_sub(norm_b[:], norm_b[:], norm_a[:])
                # now: norm_b = even-source part, norm_a = odd-source part

            # ---------- Phase C ----------
            apx = keep.tile([128, SLOTS_G // 16], i16, tag="apx")
            nc.sync.dma_start(out=apx[:], in_=apg_ix.ap())
            h0 = keep.tile([128, T8], f32, tag="h0")
            fcur = keep.tile([128, T8], f32, tag="fcur")
            red = keep.tile([128, CG], f32, tag="red")
            with tc.tile_pool(name="pc", bufs=1) as pc:
                ftab = pc.tile([128, NP], f16, tag="ftab")
                mk = pc.tile([128, T8], f32, tag="ta")
                nc.sync.dma_start(out=mk[:], in_=maskg.ap())
                nc.scalar.activation(h0[:], mk[:], AF.Relu)
                nc.vector.tensor_copy(fcur[:], h0[:])
                for it in range(K):
                    g8 = pc.tile([128, T8], f32, tag="ta")
                    nc.vector.tensor_mul(g8[:], fcur[:], dinv[:, 0:T8])
                    g16 = pc.tile([128, T8], f16, tag="tc")
                    nc.vector.tensor_copy(g16[:], g8[:])
                    for g in range(8):
                        nc.sync.dma_start(out=gblk.ap()[:, g * T8:(g + 1) * T8],
                                          in_=g16[16 * g:16 * g + 1, :])
                    nc.gpsimd.collective_compute(
                        "AllGather", mybir.AluOpType.bypass,
                        replica_groups=[list(range(NC))],
                        ins=[gblk.ap()], outs=[gath.ap()])
                    gf = gath.ap().rearrange("a b -> (a b)")
                    nc.sync.dma_start(out=ftab[0:1, :], in_=gf.unsqueeze(0))
                    for st in range(7):
                        w_ = 1 << st
                        nc.sync.dma_start(out=ftab[w_:2 * w_, :],
                                          in_=ftab[0:w_, :])
                    SP = SLOTS_G // NPASS
                    CP = CG // NPASS
                    for ps in range(NPASS):
                        sl0 = ps * SP
                        ag = pc.tile([128, SP, 2], f16, tag="ag")
                        nc.gpsimd.ap_gather(
                            out_ap=ag[:],
                            in_ap=ftab[:].rearrange("p (a b) -> p a b", b=2),
                            idxs_ap=apx[:, sl0 // 16:(sl0 + SP) // 16],
                            channels=128, num_elems=NP // 2, d=2, num_idxs=SP)
                        msa = pc.tile([128, SP], f16, tag="msa")
                        nc.vector.tensor_mul(msa[:], ag[:, :, 0],
                                             norm_b[:, sl0:sl0 + SP])
                        msb = pc.tile([128, SP], f16, tag="msb")
                        nc.vector.tensor_mul(msb[:], ag[:, :, 1],
                                             norm_a[:, sl0:sl0 + SP])
                        nc.vector.tensor_add(msa[:], msa[:], msb[:])
                        nc.vector.tensor_reduce(
                            out=red[:, ps * CP:(ps + 1) * CP]
                            .rearrange("p (a o) -> p a o", o=1),
                            in_=msa[:].rearrange("p (a w) -> p a w", w=W8),
                            axis=mybir.AxisListType.X, op=mybir.AluOpType.add)
                    for r in range(2, max_vr + 1):
                        if O[r] == 0:
                            continue
                        tb = tier_base[r]
                        nc.vector.tensor_add(red[:, 0:O[r]], red[:, 0:O[r]],
                                             red[:, tb:tb + O[r]])
                    t1 = pc.tile([128, T8], f32, tag="ta")
                    nc.vector.tensor_scalar_mul(t1[:], red[:, 0:T8], co_t[:, 1:2])
                    t2 = pc.tile([128, T8], f32, tag="tb")
                    nc.vector.tensor_scalar_mul(t2[:], h0[:], co_t[:, 0:1])
                    nc.vector.tensor_add(fcur[:], t1[:], t2[:])
                fm = pc.tile([128, T8], f32, tag="ta")
                nc.vector.tensor_scalar_sub(fm[:], fcur[:], co_t[:, 2:3])
                fo = pc.tile([128, T8], f32, tag="tb")
                nc.scalar.activation(fo[:], fm[:], mybir.ActivationFunctionType.Tanh)
                for g in range(8):
                    nc.sync.dma_start(out=fill_out.ap()[:, g * T8:(g + 1) * T8],
                                      in_=fo[16 * g:16 * g + 1, :])
    nc.compile()
    return nc


def kernel(x, mask, edge_index, W, b_lin, alpha, bias):
    x = np.asarray(x, np.float32)
    mask = np.asarray(mask, np.float32)
    W = np.asarray(W, np.float32)
    b_lin = np.asarray(b_lin, np.float32)
    alpha_v = float(np.asarray(alpha))
    bias_v = float(np.asarray(bias, np.float32).reshape(-1)[0])
    ei = np.asarray(edge_index, np.int64)
    row, col = ei[0], ei[1]
    nloc_real = N // NC

    core_of = np.minimum(col // nloc_real, NC - 1)

    # ---- per-core ELL construction ----
    core_data = []
    for k in range(NC):
        sel = np.nonzero(core_of == k)[0]
        tg = col[sel] - k * nloc_real
        order = np.argsort(tg, kind="stable")
        e_idx = sel[order]
        srcs = row[sel][order]
        tg = tg[order]
        counts = np.bincount(tg, minlength=nloc_real)
        vrows = (counts + 1 + W8 - 1) // W8
        gsz = int(np.ceil(nloc_real / 8))
        tgrp = np.minimum(np.arange(nloc_real) // gsz, 7)
        gj = np.zeros(nloc_real, np.int64)
        o_counts = np.zeros((8, 8), np.int64)
        max_vr_k = int(vrows.max())
        for g in range(8):
            tids = np.nonzero(tgrp == g)[0]
            og = tids[np.argsort(-vrows[tids], kind="stable")]
            gj[og] = np.arange(len(og))
            for r in range(2, max_vr_k + 1):
                o_counts[g, r] = int((vrows[tids] >= r).sum())
        core_data.append(dict(e_idx=e_idx, srcs=srcs, tg=tg, counts=counts,
                              vrows=vrows, tgrp=tgrp, gj=gj,
                              o_counts=o_counts, max_vr=max_vr_k))

    max_vr = max(c["max_vr"] for c in core_data)
    O = [0, 0] + [int(max(c["o_counts"][:, r].max() for c in core_data))
                  for r in range(2, max_vr + 1)]
    tier_base = {}
    cb = 832
    for r in range(2, max_vr + 1):
        tier_base[r] = cb
        cb += O[r]
    CG = -(-cb // (8 * NPASS)) * (8 * NPASS)
    SLOTS_G = CG * W8
    EGRID = 8 * SLOTS_G

    # global renumber: node n -> padded id
    glob_renum = np.zeros(N, np.int64)
    for k in range(NC):
        c = core_data[k]
        glob_renum[k * nloc_real:(k + 1) * nloc_real] = \
            k * NLOC + c["tgrp"] * T8 + c["gj"]
    assert not np.any(glob_renum == ZR_LO)

    # x in padded transposed layout (identical for all cores)
    x_rows = np.zeros((NP, D), np.float32)
    x_rows[glob_renum] = x
    xT_pad = np.ascontiguousarray(x_rows.T)

    coefs = np.zeros((128, 4), np.float32)
    coefs[:, 0] = alpha_v
    coefs[:, 1] = 1.0 - alpha_v
    coefs[:, 2] = float(np.log1p(np.exp(bias_v)))

    run_maps = []
    slot_pos_all = []
    for k in range(NC):
        c = core_data[k]
        src_grid = np.full((8, CG, W8), -1, np.int64)
        pos_grid = np.full((8, CG, W8), -1, np.int64)
        start = np.zeros(nloc_real + 1, np.int64)
        np.cumsum(c["counts"], out=start[1:])
        srcs_pad = glob_renum[c["srcs"]]
        for t in range(nloc_real):
            g = c["tgrp"][t]
            j = c["gj"][t]
            own = glob_renum[k * nloc_real + t]
            ls = np.concatenate(([own], srcs_pad[start[t]:start[t + 1]]))
            lp = np.concatenate(([-1], c["e_idx"][start[t]:start[t + 1]]))
            nv = (len(ls) + W8 - 1) // W8
            for r in range(nv):
                colc = j if r == 0 else tier_base[r + 1] + j
                seg_s = ls[r * W8:(r + 1) * W8]
                seg_p = lp[r * W8:(r + 1) * W8]
                src_grid[g, colc, :len(seg_s)] = seg_s
                pos_grid[g, colc, :len(seg_p)] = seg_p
        src_flat = src_grid.reshape(-1)
        pos_flat = pos_grid.reshape(-1)
        slot_pos_all.append(pos_flat)

        pad = src_flat < 0
        src_z = np.where(pad, ZR_HI, src_flat)       # pads -> zero row (hi)
        is_lo = src_z < 32768
        hs_lo_v = np.where(is_lo, src_z, ZR_LO)
        hs_hi_v = np.where(is_lo, ZR_HI - 32768, src_z - 32768)

        tgt_col = np.zeros(CG, np.int64)
        tgt_col[0:T8] = np.arange(T8)
        for r in range(2, max_vr + 1):
            if O[r] == 0:
                continue
            tb = tier_base[r]
            tgt_col[tb:tb + O[r]] = np.arange(O[r])
        hd_grid = np.zeros((8, CG, W8), np.int64)
        for g in range(8):
            hd_grid[g] = (k * NLOC + g * T8 + tgt_col)[:, None]
        hd_flat = np.where(pad, ZR_HI, hd_grid.reshape(-1))
        hd_islo = hd_flat < 32768
        hd_lo_v = np.where(hd_islo, hd_flat, ZR_LO)
        hd_hi_v = np.where(hd_islo, ZR_HI - 32768, hd_flat - 32768)

        par = np.where(pad, 0, src_z % 2).astype(np.float16)
        pair = np.where(pad, 0, src_z // 2)
        pmf = np.zeros((128, SLOTS_G), np.float16)
        pv = par.reshape(8, SLOTS_G)
        for g in range(8):
            pmf[16 * g:16 * (g + 1), :] = pv[g][None, :]

        mg = np.zeros((128, T8), np.float32)
        mvals = mask[k * nloc_real:(k + 1) * nloc_real, 0]
        for g in range(8):
            s = c["tgrp"] == g
            mg[16 * g:16 * (g + 1), c["gj"][s]] = mvals[s]

        run_maps.append({
            "xT": xT_pad, "WT": np.ascontiguousarray(W.T),
            "bL": b_lin.reshape(128, 1).astype(np.float32), "coefs": coefs,
            "maskg": mg,
            "hs_lo": _wrap16(hs_lo_v, EGRID), "hs_hi": _wrap16(hs_hi_v, EGRID),
            "hd_lo": _wrap16(hd_lo_v, EGRID), "hd_hi": _wrap16(hd_hi_v, EGRID),
            "apg_ix": _wrapg(pair, SLOTS_G), "par_m": pmf,
        })

    key = (CG, tuple(O), max_vr)
    if key not in _prog_cache:
        _prog_cache[key] = _build_program(CG, O, tier_base, max_vr)
    nc = _prog_cache[key]

    res = run_bass_kernel_spmd(nc, run_maps, core_ids=list(range(NC)))

    edge_weights = np.zeros(E, np.float32)
    fill = np.zeros((N, 1), np.float32)
    for k in range(NC):
        w_flat = res.results[k]["w_out"].reshape(-1)
        pos = slot_pos_all[k]
        sel = pos >= 0
        edge_weights[pos[sel]] = w_flat[sel]
        f_loc = res.results[k]["fill_out"].reshape(-1)
        c = core_data[k]
        loc = c["tgrp"] * T8 + c["gj"]
        fill[k * nloc_real:(k + 1) * nloc_real, 0] = f_loc[loc]
    return fill, edge_weights
